# revision 23
# baseline (speedup 1.0000x reference)
"""Trainium2 Bass kernel for fused QKV projection + per-head spatial attention.

Problem shapes (hardcoded from the task spec):
  x:        (2, 1024, 64, 512) fp32
  w_qkv:    (1536, 512) fp32   -> q|k|v each 512 feats = 8 heads x 64
  pos_bias: (8, 64, 64) fp32
  focus_present_mask: (2,) bool

Algorithm notes:
  - For a batch with focus_present_mask=True the mask is the identity ->
    softmax(sim masked to diag) == I exactly -> out = V = x @ w_v.T.
    Those positions only need the V projection.
  - For unfocused batches: full attention with pos_bias, no masking
    (mask is all-ones). Values are O(1) so exp() without amax shift is
    safe in fp32.
  - Sharding: data-parallel over (b*hw) positions across 8 cores.

Device kernel layout choices (no on-device transposes):
  - x is transposed on host -> xT (512, tokens): contraction dim on
    partitions for every matmul.
  - q^T,k^T produced feature-major (128 part = 2 heads x 64 dim), which
    directly feeds sim^T = (k^T).T @ q^T   (out: j on partitions, i free).
  - V produced token-major (128 part = 2 positions x 64 tokens) with a
    ones column appended, so PV = E.T @ [V|1] yields the softmax
    denominator as a per-partition column -> native tensor_tensor
    broadcast normalize.
  - sim^T for 16 (position, head) pairs packed into one PSUM bank pair;
    the pos-bias add is one tensor_tensor (GpSimd) against a
    host-precomputed (128, 512) constant; exp is one ScalarE activation
    per 16 pairs.
  - PV uses all 4 PE quadrants: out rows keyed by head-half (hb), PSUM
    bank keyed by position parity (p2); the output DMA un-permutes.
  - Attention chunks and V-only chunks are interleaved 1:1 so the
    DMA-heavy V-projection work rides under the compute-heavy attention
    work (keeps PE warm, hides stores).
  - All outputs stored bf16 (halves store traffic), cast to fp32 on host.
"""

import numpy as np

import concourse.bass as bass
import concourse.bacc as bacc
import concourse.mybir as mybir
import concourse.tile as tile
from concourse.bass_utils import run_bass_kernel_spmd

HEADS = 8
DH = 64
NTOK = 64          # tokens per spatial position
DIM = 512
QK_FEATS = 1024    # q + k feature columns
N_CORES = 8
P = 128
F32 = mybir.dt.float32
BF16 = mybir.dt.bfloat16

# test.py introspection: last BassKernelResults (exec_time_ns when BASS_TRACE=1)
LAST_RESULT = None

_KERNEL_CACHE: dict = {}


def _ensure_ntff_hook():
    """Make BASS_TRACE=1 usable: bass_utils' axon trace path imports
    antenv.axon_hooks, which some images lack. Provide the tiny get/set
    pair and register the ctypes NTFF hook the boot code would have."""
    import sys
    import types

    try:
        import antenv.axon_hooks  # noqa: F401

        return
    except ImportError:
        pass
    try:
        import antenv
        from trn_agent_boot.trn_boot import _ntff_profile_via_ctypes
    except ImportError:
        return
    mod = types.ModuleType("antenv.axon_hooks")
    _state = {"hook": None}
    mod.set_axon_ntff_profile_hook = lambda h: _state.__setitem__("hook", h)
    mod.get_axon_ntff_profile_hook = lambda: _state["hook"]
    sys.modules["antenv.axon_hooks"] = mod
    antenv.axon_hooks = mod
    import os as _os

    so = "/opt/axon/libaxon_pjrt.so"
    if _os.path.exists(so):
        try:
            mod.set_axon_ntff_profile_hook(_ntff_profile_via_ctypes(so))
        except Exception:
            pass


def _ecol(h):
    # exp-space column of head h: bank by head parity, then by h//2
    return (h % 2) * 256 + (h // 2) * 64


def _build_kernel(a_tok: int, v_tok: int):
    """Build the per-core Bass program.

    a_tok: tokens needing full attention on this core (multiple of 512, may be 0)
    v_tok: tokens needing only the V projection (multiple of 512, may be 0)
    """
    nc = bacc.Bacc("TRN2")

    wqkT = nc.dram_tensor("wqkT", [DIM, QK_FEATS], BF16, kind="ExternalInput")
    wvT = nc.dram_tensor("wvT", [DIM, DIM], BF16, kind="ExternalInput")
    ebiasT = nc.dram_tensor("ebiasT", [P, 512], BF16, kind="ExternalInput")
    xaT = out_a = None
    if a_tok:
        xaT = nc.dram_tensor("xaT", [DIM, a_tok], BF16, kind="ExternalInput")
        out_a = nc.dram_tensor("out_a", [a_tok, DIM], BF16, kind="ExternalOutput")
    xvT = out_v = None
    if v_tok:
        xvT = nc.dram_tensor("xvT", [DIM, v_tok], BF16, kind="ExternalInput")
        out_v = nc.dram_tensor("out_v", [v_tok, DIM], BF16, kind="ExternalOutput")

    EXP = mybir.ActivationFunctionType.Exp

    with tile.TileContext(nc) as tc:
        with tc.tile_pool(name="const", bufs=1) as const:
            # wv first: the V-projection is the first consumer
            wv_sb = const.tile([P, 4, DIM], BF16)
            wvT_r = wvT[:, :].rearrange("(k p) e -> p k e", p=P)
            # split by contraction tile so the very first matmul can start
            # as soon as its slice lands
            for kt in range(4):
                nc.sync.dma_start(wv_sb[:, kt], wvT_r[:, kt])
            # wqk/ebias tiles created here, but their loads are issued after
            # chunk 0's input DMA (they are needed a few us later)
            wqk_sb = const.tile([P, 4, QK_FEATS], BF16)
            ebias_sb = const.tile([P, 512], BF16)

            na = a_tok // 512
            nv = v_tok // 512
            with (
                tc.tile_pool(name="ax", bufs=4) as xpool,
                tc.tile_pool(name="aqk", bufs=2) as qkpool,
                tc.tile_pool(name="av", bufs=1) as vpool,
                tc.tile_pool(name="ae", bufs=4) as epool,
                tc.tile_pool(name="ao", bufs=2) as opool,
                tc.tile_pool(name="aov", bufs=2) as ovpool,
                tc.tile_pool(name="ar", bufs=8) as rpool,
                tc.tile_pool(name="app", bufs=2, space="PSUM") as pp_proj,
                tc.tile_pool(name="aps", bufs=1, space="PSUM") as pp_s,
                tc.tile_pool(name="apo", bufs=2, space="PSUM") as pp_o,
            ):
                # software pipeline: chunk c+1's projections are emitted
                # interleaved with chunk c's attention groups, so each
                # engine queue alternates between PSUM-drain copies (which
                # unblock the PE) and the group chain ops (exp/e_mult) that
                # gate the PV matmuls.
                staged = None
                if na:
                    # chunk 0 input: per-kt slices, issued before wqk so the
                    # critical first bytes share DMA bandwidth with fewer
                    # competitors
                    xT0 = xpool.tile([P, 4, 512], BF16, tag="xT")
                    xaT_r0 = xaT[:, :].rearrange("(k p) t -> p k t", p=P)
                    for kt in range(4):
                        nc.sync.dma_start(xT0[:, kt], xaT_r0[:, kt, 0:512])
                nc.sync.dma_start(
                    wqk_sb[:], wqkT[:, :].rearrange("(k p) e -> p k e", p=P)
                )
                nc.sync.dma_start(ebias_sb[:], ebiasT[:, :])
                if na:
                    vts0, _ = _proj_v(
                        nc, 0, xaT, wv_sb, xpool, vpool, pp_proj, xT_pre=xT0
                    )
                    qkT0 = _proj_qk(nc, wqk_sb, qkpool, pp_proj, xT0, 0, 8)
                    staged = (vts0, qkT0)
                for c in range(max(na, nv)):
                    if c >= na:
                        _v_chunk(
                            nc, c, xvT, out_v, wv_sb, xpool, ovpool, pp_proj
                        )
                        continue
                    vts, qkT = staged
                    nxt_v = nxt_x = qkT_n = None
                    if c + 1 < na:
                        nxt_v, nxt_x = _proj_v(
                            nc, c + 1, xaT, wv_sb, xpool, vpool, pp_proj
                        )
                    ot = opool.tile([P, 2048], BF16, tag="ot")
                    _group(nc, 0, ebias_sb, vts, qkT, ot,
                           epool, rpool, pp_s, pp_o, EXP)
                    if c + 1 < na:
                        qkT_n = _proj_qk(
                            nc, wqk_sb, qkpool, pp_proj, nxt_x, 0, 4
                        )
                    _group(nc, 1, ebias_sb, vts, qkT, ot,
                           epool, rpool, pp_s, pp_o, EXP)
                    if c + 1 < na:
                        _proj_qk(
                            nc, wqk_sb, qkpool, pp_proj, nxt_x, 4, 8,
                            qkT=qkT_n,
                        )
                    _group(nc, 2, ebias_sb, vts, qkT, ot,
                           epool, rpool, pp_s, pp_o, EXP)
                    if c < nv:
                        _v_chunk(
                            nc, c, xvT, out_v, wv_sb, xpool, ovpool, pp_proj
                        )
                    _group(nc, 3, ebias_sb, vts, qkT, ot,
                           epool, rpool, pp_s, pp_o, EXP)
                    # out rows for chunk c: token rows c*512 + g*128 + p2*64
                    # + t, features hb*256 + cc; ot = [part=(hb,t),
                    # col=(g,p2,cc)]
                    row0 = c * 512
                    for hb in range(2):
                        src = ot[hb * 64 : (hb + 1) * 64, :].rearrange(
                            "t (g p2 cc) -> t g p2 cc", g=4, p2=2
                        )
                        dst = out_a[
                            row0 : row0 + 512, hb * 256 : (hb + 1) * 256
                        ].rearrange("(g p2 t) cc -> t g p2 cc", g=4, p2=2)
                        nc.sync.dma_start(dst, src)
                    staged = (nxt_v, qkT_n)

    nc.finalize()
    return nc


def _proj_v(nc, c, xaT, wv_sb, xpool, vpool, pp_proj, xT_pre=None):
    if xT_pre is not None:
        xT = xT_pre
    else:
        xaT_r = xaT[:, :].rearrange("(k p) t -> p k t", p=P)
        xT = xpool.tile([P, 4, 512], BF16, tag="xT")
        nc.sync.dma_start(xT[:], xaT_r[:, :, c * 512 : (c + 1) * 512])

    # --- V projection first: its lhsT=xT ldweights absorbs the DMA wait
    vts = []
    for tt in range(4):
        psv = pp_proj.tile([P, 512], F32, tag="ps_proj")
        for kt in range(4):
            nc.tensor.matmul(
                psv[:],
                lhsT=xT[:, kt, tt * 128 : (tt + 1) * 128],
                rhs=wv_sb[:, kt, :],
                start=(kt == 0),
                stop=(kt == 3),
            )
        # persistent ring slot: the ones-column written on first use survives
        vt = vpool.tile([P, 8, 65], BF16, tag=f"vt{(c % 2) * 4 + tt}")
        if c < 2:
            nc.gpsimd.memset(vt[:, :, 64:65], 1.0)
        # alternate the PSUM-drain engine so neither queue backs up
        if tt % 2 == 0:
            nc.vector.tensor_copy(
                out=vt[:, :, 0:64],
                in_=psv[:].rearrange("p (h d) -> p h d", h=8),
            )
        else:
            nc.scalar.copy(
                out=vt[:, :, 0:64],
                in_=psv[:].rearrange("p (h d) -> p h d", h=8),
            )
        vts.append(vt)
    return vts, xT


def _proj_qk(nc, wqk_sb, qkpool, pp_proj, xT, ft_lo, ft_hi, qkT=None):
    # q^T, k^T projection: feature-major (2 heads per 128 partitions)
    if qkT is None:
        qkT = qkpool.tile([P, 8, 512], BF16, tag="qkT")
    for ft in range(ft_lo, ft_hi):
        ps = pp_proj.tile([P, 512], F32, tag="ps_proj")
        for kt in range(4):
            nc.tensor.matmul(
                ps[:],
                lhsT=wqk_sb[:, kt, ft * 128 : (ft + 1) * 128],
                rhs=xT[:, kt, :],
                start=(kt == 0),
                stop=(kt == 3),
            )
        nc.scalar.copy(out=qkT[:, ft, :], in_=ps[:])
    return qkT


def _group(nc, g, ebias_sb, vts, qkT, ot, epool, rpool, pp_s, pp_o, EXP):
    # --- attention, one group of 2 positions (16 (pos,head) pairs)
    # Concurrent matmuls on different PE row-groups must write
    # different PSUM banks (HW hang otherwise):
    #  - sim MMs: row-group = head parity -> 2-bank pss tile, bank by h%2
    #  - PV MMs: row-group = position parity (p2) -> bank by p2; out rows
    #    keyed by head-half (hb) so all 4 quadrants run concurrently.
    pss = pp_s.tile([P, 1024], F32, tag="ps_s")
    for h in range(8):
        ft = h // 2
        pb = (h % 2) * 64
        col0 = (h % 2) * 512 + (h // 2) * 64
        for p2 in range(2):
            tok0 = g * 128 + p2 * 64
            nc.tensor.matmul(
                pss[p2 * 64 : (p2 + 1) * 64, col0 : col0 + 64],
                lhsT=qkT[pb : pb + 64, 4 + ft, tok0 : tok0 + 64],
                rhs=qkT[pb : pb + 64, ft, tok0 : tok0 + 64],
                start=True,
                stop=True,
                tile_position=(pb, p2 * 64),
            )
    # exp(sim + bias) = exp(sim) * exp(bias); bias folded as a
    # multiplicative constant so pss is read by ScalarE only.
    # E col layout: ecol(h) = (h%2)*256 + (h//2)*64
    e_raw = epool.tile([P, 512], BF16, tag="Eraw")
    nc.scalar.activation(
        e_raw[:].rearrange("p (b c) -> p b c", b=2),
        pss[:].rearrange("p (b c) -> p b c", b=2)[:, :, 0:256],
        EXP,
    )
    e_t = epool.tile([P, 512], BF16, tag="E")
    nc.vector.tensor_tensor(
        e_t[:], e_raw[:], ebias_sb[:], mybir.AluOpType.mult
    )

    pvt = pp_o.tile([P, 1024], F32, tag="pvt")
    vt = vts[g]
    for h in range(8):
        hb, hh = h // 4, h % 4
        for p2 in range(2):
            nc.tensor.matmul(
                pvt[hb * 64 : (hb + 1) * 64,
                    p2 * 512 + hh * 65 : p2 * 512 + hh * 65 + 65],
                lhsT=e_t[p2 * 64 : (p2 + 1) * 64, _ecol(h) : _ecol(h) + 64],
                rhs=vt[p2 * 64 : (p2 + 1) * 64, h, :],
                start=True,
                stop=True,
                tile_position=(p2 * 64, hb * 64),
            )

    # normalize: denominators sit at col 64 of each 65-block
    pvt_r = (
        pvt[:]
        .rearrange("p (p2 c) -> p p2 c", p2=2)[:, :, 0:260]
        .rearrange("p p2 (h x) -> p p2 h x", h=4)
    )
    rec = rpool.tile([P, 2, 4, 1], F32, tag="rec")
    nc.vector.reciprocal(rec[:], pvt_r[:, :, :, 64:65])
    nc.vector.tensor_tensor(
        ot[:, g * 512 : (g + 1) * 512].rearrange(
            "p (p2 h d) -> p p2 h d", p2=2, h=4
        ),
        pvt_r[:, :, :, 0:64],
        rec[:].to_broadcast((P, 2, 4, 64)),
        mybir.AluOpType.mult,
    )


def _v_chunk(nc, c, xvT, out_v, wv_sb, xpool, ovpool, pp_proj):
    xvT_r = xvT[:, :].rearrange("(k p) t -> p k t", p=P)
    xT = xpool.tile([P, 4, 512], BF16, tag="xT2")
    nc.sync.dma_start(xT[:], xvT_r[:, :, c * 512 : (c + 1) * 512])
    ov = ovpool.tile([P, 4, 512], BF16, tag="ov")
    for tt in range(4):
        psv = pp_proj.tile([P, 512], F32, tag="ps_proj")
        for kt in range(4):
            nc.tensor.matmul(
                psv[:],
                lhsT=xT[:, kt, tt * 128 : (tt + 1) * 128],
                rhs=wv_sb[:, kt, :],
                start=(kt == 0),
                stop=(kt == 3),
            )
        if tt % 2 == 0:
            nc.vector.tensor_copy(out=ov[:, tt, :], in_=psv[:])
        else:
            nc.scalar.copy(out=ov[:, tt, :], in_=psv[:])
        # store each 128-row block as soon as its copy lands
        nc.sync.dma_start(
            out_v[c * 512 + tt * 128 : c * 512 + (tt + 1) * 128, :],
            ov[:, tt, :],
        )


def _pad_positions(idx: np.ndarray, mult: int) -> np.ndarray:
    """Pad a position-index list to a multiple of `mult` by repeating the last
    entry (duplicates are recomputed and harmlessly overwritten on scatter)."""
    if len(idx) % mult == 0:
        return idx
    pad = mult - len(idx) % mult
    return np.concatenate([idx, np.full(pad, idx[-1], dtype=idx.dtype)])


def host_consts(w_qkv, pos_bias):
    """Host-side constant prep shared by kernel() and tests."""
    import ml_dtypes
    bf16 = ml_dtypes.bfloat16
    scale = DH ** -0.5
    wq = w_qkv[0:512] * scale
    wk = w_qkv[512:1024]
    wv = w_qkv[1024:1536]
    wqkT = np.ascontiguousarray(np.concatenate([wq, wk], axis=0).T.astype(bf16))
    wvT = np.ascontiguousarray(wv.T.astype(bf16))
    # ebiasT[p2*64+j, ecol(h)+i] = exp(pos_bias[h, i, j]), ecol = (h%2)*256+(h//2)*64
    big = np.zeros((64, 512), np.float32)
    for h in range(HEADS):
        big[:, _ecol(h) : _ecol(h) + 64] = pos_bias[h].T
    ebiasT = np.ascontiguousarray(np.exp(np.tile(big, (2, 1))).astype(bf16))
    return wqkT, wvT, ebiasT


def kernel(x, w_qkv, pos_bias, focus_present_mask):
    global LAST_RESULT
    _ensure_ntff_hook()
    x = np.ascontiguousarray(np.asarray(x), dtype=np.float32)
    w_qkv = np.asarray(w_qkv, dtype=np.float32)
    pos_bias = np.asarray(pos_bias, dtype=np.float32)
    mask = np.asarray(focus_present_mask).astype(bool)

    b, hw, n, dim = x.shape
    assert (n, dim) == (NTOK, DIM) and w_qkv.shape == (3 * HEADS * DH, DIM)
    x_flat = x.reshape(b * hw, n, dim)

    flat_idx = np.arange(b * hw)
    batch_of = flat_idx // hw
    attn_idx = flat_idx[~mask[batch_of]]
    vpr_idx = flat_idx[mask[batch_of]]

    # per-core granularity: 8 positions (one 512-token chunk) x 8 cores
    attn_idx = _pad_positions(attn_idx, 8 * N_CORES) if len(attn_idx) else attn_idx
    vpr_idx = _pad_positions(vpr_idx, 8 * N_CORES) if len(vpr_idx) else vpr_idx
    a_pos_pc = len(attn_idx) // N_CORES
    v_pos_pc = len(vpr_idx) // N_CORES
    a_tok = a_pos_pc * NTOK
    v_tok = v_pos_pc * NTOK

    key = (a_tok, v_tok)
    if key not in _KERNEL_CACHE:
        _KERNEL_CACHE[key] = _build_kernel(a_tok, v_tok)
    nc = _KERNEL_CACHE[key]

    import ml_dtypes
    bf16 = ml_dtypes.bfloat16
    wqkT, wvT, ebiasT = host_consts(w_qkv, pos_bias)

    in_maps = []
    for core in range(N_CORES):
        m = {"wqkT": wqkT, "wvT": wvT, "ebiasT": ebiasT}
        if a_tok:
            ai = attn_idx[core * a_pos_pc : (core + 1) * a_pos_pc]
            m["xaT"] = np.ascontiguousarray(x_flat[ai].reshape(-1, DIM).T.astype(bf16))
        if v_tok:
            vi = vpr_idx[core * v_pos_pc : (core + 1) * v_pos_pc]
            m["xvT"] = np.ascontiguousarray(x_flat[vi].reshape(-1, DIM).T.astype(bf16))
        in_maps.append(m)

    res = run_bass_kernel_spmd(nc, in_maps, core_ids=list(range(N_CORES)))
    LAST_RESULT = res

    out_flat = np.empty((b * hw, n, HEADS * DH), dtype=np.float32)
    for core in range(N_CORES):
        if a_tok:
            ai = attn_idx[core * a_pos_pc : (core + 1) * a_pos_pc]
            out_flat[ai] = (
                res.results[core]["out_a"]
                .astype(np.float32)
                .reshape(a_pos_pc, n, HEADS * DH)
            )
        if v_tok:
            vi = vpr_idx[core * v_pos_pc : (core + 1) * v_pos_pc]
            out_flat[vi] = (
                res.results[core]["out_v"]
                .astype(np.float32)
                .reshape(v_pos_pc, n, HEADS * DH)
            )
    return out_flat.reshape(b, hw, n, HEADS * DH)
